# revision 33
# baseline (speedup 1.0000x reference)
"""Sparse block-attention kernel for Trainium2, 8 NeuronCores.

Problem: y = proj(softmax_masked((x@Wq)(x@Wk)^T/8) @ (x@Wv)) for
B=2, L=3392 (=3*1024 motion + 5*64 text), C=512, H=8 heads, dh=64.

Sharding: 16 (batch, head) pairs over 8 cores -> each core owns one batch
and two adjacent heads (h0=2*(core%4), h1=h0+1). Each core computes its
heads' QKV projections, masked attention, and the partial output
projection y_h0@Wp_h0 + y_h1@Wp_h1 [L, C]; the host sums the 4 partials
per batch.

On-core layout: scores are computed transposed, ST = k q^T [keys, queries],
with both heads packed into the 128 partitions via tensor-engine row
tiling (head0 uses array rows 0-63, head1 rows 64-127). The softmax
denominator comes free from a ones-column appended to V in the AV matmul.
Scores are ~N(0,1) so exp() needs no max subtraction. The block-causal
mask is realized by iterating only allowed key tiles, with trapezoid
query slices and gpsimd affine_select zeroing of diagonal corners
after exp.
"""

import numpy as np
import ml_dtypes

import concourse.bass as bass
import concourse.bacc as bacc
import concourse.mybir as mybir
import concourse.tile as tile
from concourse.bass_utils import run_bass_kernel_spmd
from concourse.masks import make_identity

T = 1024
NTX = 64
L = 3 * T + 5 * NTX  # 3392
C = 512
H = 8
DH = 64
NCORES = 8
N_BLOCKS = 7
BLOCK_Q0 = [0, 512, 1024, 1536, 2048, 2560, 3072]
BLOCK_W = [512, 512, 512, 512, 512, 512, 320]
CHW = [512, 512, 512, 512, 512, 512, 320]  # L chunks (same grid as blocks)
NSUB = [4, 4, 4, 4, 4, 4, 3]  # 128-row subtiles per chunk (last: 128,128,64)
BF16 = mybir.dt.bfloat16
F32 = mybir.dt.float32
NPBF16 = np.dtype(ml_dtypes.bfloat16)


def key_entries(block):
    """Allowed key tiles for a query block: (kstart, klen, qlo, mask).

    The QK/AV matmuls for the tile cover block queries [qlo, W); mask
    in {None,'c','s'} marks a causal/strict triangle on the first 128
    query columns of that slice.
    """
    q0 = BLOCK_Q0[block]
    w = BLOCK_W[block]
    if q0 >= 3 * T:  # text queries attend to all text keys
        return [(3072, 128, 0, None), (3200, 128, 0, None), (3328, 64, 0, None)]
    band = q0 // T
    r0 = q0 % T
    if band == 0:
        specs = [(0, 'c')]
    elif band == 1:
        specs = [(0, 'c'), (T, 's'), (2 * T, 's')]
    else:
        specs = [(0, 'c'), (T, 'c'), (2 * T, 's')]
    out = []
    for kbase, kind in specs:
        for kt in range((r0 + w) // 128):
            local = kt * 128
            if local < r0:
                out.append((kbase + local, 128, 0, None))
            else:
                out.append((kbase + local, 128, local - r0, kind))
    if band == 1:  # text domains 0,2,3,4
        out += [(3072, 64, 0, None), (3200, 128, 0, None), (3328, 64, 0, None)]
    elif band == 2:  # text domains 1,2,3,4
        out += [(3136, 64, 0, None), (3200, 128, 0, None), (3328, 64, 0, None)]
    return out


def _build_nc():
    from contextlib import ExitStack

    nc = bacc.Bacc("TRN2", target_bir_lowering=False, num_devices=NCORES)

    xT_d = nc.dram_tensor("xT", [C, L], BF16, kind="ExternalInput").ap()
    wqk_d = nc.dram_tensor("wqk", [128, 4, 2, 128], BF16, kind="ExternalInput").ap()
    wv_d = nc.dram_tensor("wv", [128, 4, 2, DH], BF16, kind="ExternalInput").ap()
    wp_d = nc.dram_tensor("wp", [128, C], BF16, kind="ExternalInput").ap()
    out_d = nc.dram_tensor("out", [L, C], F32, kind="ExternalOutput").ap()

    with tile.TileContext(nc) as tc:
        with ExitStack() as ctx:
            singles = ctx.enter_context(tc.tile_pool(name="singles", bufs=1))
            psum_st = ctx.enter_context(
                tc.tile_pool(name="psum_st", bufs=2, space="PSUM"))
            # yt accumulators and mm scratch share one 4-slot pool of
            # single-bank tiles, so two attention blocks can be in flight
            psum_mm = ctx.enter_context(
                tc.tile_pool(name="psum_mm", bufs=4, space="PSUM"))
            psum_yt = psum_mm
            ssb_pool = ctx.enter_context(tc.tile_pool(name="ssb", bufs=8))
            norm_pool = ctx.enter_context(tc.tile_pool(name="norm", bufs=3))
            out_pool = ctx.enter_context(tc.tile_pool(name="outs", bufs=3))

            # ---- weights ----
            wqk = singles.tile([128, 4, 2, 128], BF16, tag="wqk")
            wv = singles.tile([128, 4, 2, DH], BF16, tag="wv")
            wp = singles.tile([128, C], BF16, tag="wp")
            nc.sync.dma_start(out=wqk, in_=wqk_d)

            # ---- constants: 0/1 corner masks (causal, strict) + ones ----
            mcaus = singles.tile([128, 2, 128], BF16, tag="mcaus")
            mstrict = singles.tile([128, 2, 128], BF16, tag="mstrict")
            ones64 = singles.tile([1, 64], F32, tag="ones64")
            nc.gpsimd.memset(ones64, 1.0)
            ident = singles.tile([128, 128], BF16, tag="ident")
            make_identity(nc, ident)
            for mt, base in ((mcaus, 0), (mstrict, -1)):
                nc.gpsimd.memset(mt, 1.0)
                nc.gpsimd.affine_select(
                    out=mt, in_=mt, compare_op=mybir.AluOpType.is_ge,
                    fill=0.0, base=base, pattern=[[0, 2], [1, 128]],
                    channel_multiplier=-1)

            # ---- per-chunk persistent tensors ----
            xt = []   # x^T chunks [128, 4, w]
            sbA = []  # [qT_h0 (parts 0-63); kT_h1 (parts 64-127)] per chunk
            sbB = []  # [kT_h0; qT_h1]
            vaug = []  # [128, g, 130]: cols 0-63 v_h0, 64 ones, 65-128 v_h1, 129 ones
            for c in range(N_BLOCKS):
                w = CHW[c]
                xt.append(singles.tile([128, 4, w], BF16, tag=f"xt{c}", name=f"xt{c}"))
                sbA.append(singles.tile([128, w], BF16, tag=f"sbA{c}", name=f"sbA{c}"))
                sbB.append(singles.tile([128, w], BF16, tag=f"sbB{c}", name=f"sbB{c}"))
                vaug.append(singles.tile([128, NSUB[c], 130], BF16, tag=f"vaug{c}", name=f"vaug{c}"))

            xT_r = xT_d.rearrange("(t p) l -> p t l", p=128)
            for ct in range(4):
                nc.sync.dma_start(out=xt[0][:, ct, :], in_=xT_r[:, ct, 0:512])
            nc.sync.dma_start(out=wv, in_=wv_d)
            nc.sync.dma_start(out=wp, in_=wp_d)
            for c in range(1, N_BLOCKS):
                q0, w = BLOCK_Q0[c], CHW[c]
                nc.sync.dma_start(out=xt[c], in_=xT_r[:, :, q0:q0 + w])

            # ---- QK projections: psum[0:64]=qT/kT h-even, [64:128]=kT/qT h-odd
            def emit_projqk(c):
                w = CHW[c]
                for j, dst in ((0, sbA[c]), (1, sbB[c])):
                    ps = psum_mm.tile([128, 512], F32, tag="mm", name="ps")
                    for ct in range(4):
                        nc.tensor.matmul(
                            ps[:, 0:w], lhsT=wqk[:, ct, j, :], rhs=xt[c][:, ct, :],
                            start=(ct == 0), stop=(ct == 3))
                    nc.vector.tensor_copy(dst[:, :], ps[:, 0:w])

            # ---- V projection: weight-stationary vT matmuls, then PE
            # transposes into the [keys, dh] layout the AV matmul needs ----
            def emit_projv(c):
                w = CHW[c]
                # ones columns 64 and 129 for the softmax-denominator rows
                nc.gpsimd.memset(vaug[c][:, :, 64::65], 1.0)
                vps = psum_mm.tile([128, 512], F32, tag="mm", name="vps")
                for ct in range(4):
                    nc.tensor.matmul(
                        vps[:, 0:w], lhsT=wv[:, ct, :, :], rhs=xt[c][:, ct, :],
                        start=(ct == 0), stop=(ct == 3))
                vts = out_pool.tile([128, 512], BF16, tag="vts", name="vts")
                nc.vector.tensor_copy(vts[:, 0:w], vps[:, 0:w])
                for gi in range(NSUB[c]):
                    rows = min(128, w - gi * 128)
                    tp = psum_mm.tile([128, 128], BF16, tag="mm", name="tp")
                    nc.tensor.transpose(
                        tp[0:rows, :], vts[:, gi * 128:gi * 128 + rows], ident)
                    # scatter [v_h0 | v_h1] -> cols {0:64, 65:129}
                    dst = bass.AP(
                        tensor=vaug[c].tensor,
                        offset=vaug[c].offset + gi * 130,
                        ap=[[vaug[c].ap[0][0], rows], [65, 2], [1, 64]])
                    src = bass.AP(
                        tensor=tp.tensor, offset=tp.offset,
                        ap=[[tp.ap[0][0], rows], [64, 2], [1, 64]])
                    nc.vector.tensor_copy(dst, src)

            # ---- attention + normalize + output projection per query block.
            # Emitted in resumable slices (Tile dependencies follow trace
            # order, so a block's tiles must be emitted after the
            # projection chunks they read).
            astate = {}

            def attn_start(blk):
                astate[blk] = {
                    "yt0": psum_yt.tile([65, 512], F32, tag="mm", name="yt0"),
                    "yt1": psum_yt.tile([65, 512], F32, tag="mm", name="yt1"),
                    "entries": key_entries(blk),
                    "i": 0,
                    "pending": None,
                }

            def _emit_av(blk, item):
                # AV accumulate (row 64 of yt = softmax denominator)
                s = astate[blk]
                w = BLOCK_W[blk]
                nlast = len(s["entries"]) - 1
                i, ssb, pb, klen, qlo, cv, gi = item
                nc.tensor.matmul(
                    s["yt0"][:, qlo:w],
                    lhsT=vaug[cv][pb:pb + klen, gi, 0:65],
                    rhs=ssb[pb:pb + klen, 0, qlo:w],
                    start=(i == 0), stop=(i == nlast), tile_position=(pb, 0))
                nc.tensor.matmul(
                    s["yt1"][:, qlo:w],
                    lhsT=vaug[cv][pb:pb + klen, gi, 65:130],
                    rhs=ssb[pb:pb + klen, 1, qlo:w],
                    start=(i == 0), stop=(i == nlast), tile_position=(pb, 0))

            def attn_tiles(blk, n):
                # emit the next n key tiles (software pipeline: AV trails
                # QK/exp by one tile)
                s = astate[blk]
                w = BLOCK_W[blk]
                for (kstart, klen, qlo, mask) in s["entries"][s["i"]:s["i"] + n]:
                    i = s["i"]
                    ck, ko = kstart // 512, kstart % 512
                    g = kstart // 128
                    cv, gi = g // 4, g % 4
                    pb = kstart % 128  # 0 or 64
                    st = psum_st.tile([128, 2, 512], F32, tag="st", name="st")
                    ssb = ssb_pool.tile([128, 2, 512], BF16, tag="ssb",
                                        name="ssb")
                    # QK^T (scores transposed: [keys, queries]), heads packed
                    nc.tensor.matmul(
                        st[pb:pb + klen, 0, qlo:w],
                        lhsT=sbB[ck][0:64, ko:ko + klen],
                        rhs=sbA[blk][0:64, qlo:w],
                        start=True, stop=True, tile_position=(0, pb))
                    nc.tensor.matmul(
                        st[pb:pb + klen, 1, qlo:w],
                        lhsT=sbA[ck][64:128, ko:ko + klen],
                        rhs=sbB[blk][64:128, qlo:w],
                        start=True, stop=True, tile_position=(64, pb))
                    nc.scalar.activation(
                        ssb[pb:pb + klen, :, qlo:w], st[pb:pb + klen, :, qlo:w],
                        mybir.ActivationFunctionType.Exp)
                    if mask is not None:
                        # zero the blocked triangle of the diagonal corner
                        mt = mcaus if mask == 'c' else mstrict
                        nc.vector.tensor_mul(
                            ssb[pb:pb + klen, :, qlo:qlo + 128],
                            ssb[pb:pb + klen, :, qlo:qlo + 128],
                            mt[pb:pb + klen, :, :])
                    if s["pending"] is not None:
                        _emit_av(blk, s["pending"])
                    s["pending"] = (i, ssb, pb, klen, qlo, cv, gi)
                    s["i"] += 1

            def attn_finish(blk):
                s = astate[blk]
                w = BLOCK_W[blk]
                attn_tiles(blk, len(s["entries"]) - s["i"])
                _emit_av(blk, s["pending"])
                yt0, yt1 = s["yt0"], s["yt1"]

                # normalize: y / l, heads stacked into [128, w] bf16
                ytsb = norm_pool.tile([128, 512], BF16, tag="ytsb")
                h1t = norm_pool.tile([64, 512], BF16, tag="h1t")
                for j, (yt, mdst) in enumerate(((yt0, ytsb[0:64, 0:w]),
                                                (yt1, h1t[:, 0:w]))):
                    rc = norm_pool.tile([1, 512], F32, tag=f"rc{j}", name=f"rc{j}")
                    lsb = norm_pool.tile([1, 512], F32, tag=f"lsb{j}", name=f"lsb{j}")
                    rb = norm_pool.tile([64, 512], F32, tag=f"rb{j}", name=f"rb{j}")
                    nc.vector.tensor_copy(lsb[:, 0:w], yt[64:65, 0:w])
                    nc.vector.reciprocal_approx_fast(rc[:, 0:w], lsb[:, 0:w])
                    nc.gpsimd.partition_broadcast(rb[:, 0:w], rc[:, 0:w])
                    nc.vector.tensor_mul(mdst, yt[0:64, 0:w], rb[:, 0:w])
                # head1 rows into partitions 64-127 (partition shift via DMA)
                nc.sync.dma_start(out=ytsb[64:128, 0:w], in_=h1t[:, 0:w])

                # output projection: one K=128 matmul per 128 queries
                for sub in range(NSUB[blk]):
                    rows = min(128, w - sub * 128)
                    po = psum_mm.tile([128, 512], F32, tag="mm", name="po")
                    nc.tensor.matmul(
                        po[0:rows, :],
                        lhsT=ytsb[:, sub * 128:sub * 128 + rows],
                        rhs=wp[:, :], start=True, stop=True)
                    ost = out_pool.tile([128, 512], F32, tag="ost")
                    nc.vector.tensor_copy(ost[0:rows, :], po[0:rows, :])
                    r0 = BLOCK_Q0[blk] + sub * 128
                    nc.sync.dma_start(out=out_d[r0:r0 + rows, :],
                                      in_=ost[0:rows, :])

            def emit_attn(blk):
                attn_start(blk)
                attn_finish(blk)

            # Interleave projections with attention so the scalar engine
            # (exp, the bottleneck) starts as early as dependencies allow.
            # Big block 3 leads: its band-0 tiles need only chunks 0/1, its
            # band-1 tiles chunks 2/3, so its exp work covers the remaining
            # projections.
            emit_projqk(0)
            emit_projv(0)
            attn_start(0)
            attn_tiles(0, 4)
            emit_projqk(1)
            emit_projv(1)
            attn_finish(0)
            attn_start(1)
            attn_tiles(1, 8)
            emit_projqk(3)
            emit_projv(3)
            attn_finish(1)
            attn_start(3)
            attn_tiles(3, 8)   # band0 key tiles (keys in chunks 0-1)
            emit_projqk(2)
            emit_projv(2)
            attn_tiles(3, 8)   # band1 key tiles (keys in chunks 2-3)
            for c in (6, 4, 5):
                emit_projqk(c)
                emit_projv(c)
            attn_finish(3)
            for blk in (5, 2, 4, 6):
                emit_attn(blk)
    nc.finalize()
    return nc


_NC = None
TRACE = False
LAST = None


def _get_nc():
    global _NC
    if _NC is None:
        _NC = _build_nc()
    return _NC


def kernel(x, Wq, bq, Wk, bk, Wv, bv, Wp, bp, T_motion, N, **_unused):
    x = np.asarray(x, np.float32)
    Wq = np.asarray(Wq, np.float32)
    Wk = np.asarray(Wk, np.float32)
    Wv = np.asarray(Wv, np.float32)
    Wp = np.asarray(Wp, np.float32)
    bq = np.asarray(bq, np.float32)
    bk = np.asarray(bk, np.float32)  # provably softmax-invariant, ignored
    bv = np.asarray(bv, np.float32)
    bp = np.asarray(bp, np.float32)
    assert int(T_motion) == T and int(N) == NTX, "kernel compiled for T=1024,N=64"
    assert x.shape == (2, L, C)
    assert np.allclose(bq, 0.0), "nonzero bq not supported"

    Wq8 = Wq / 8.0
    in_maps = []
    for core in range(NCORES):
        b = core // 4
        h0 = 2 * (core % 4)
        h1 = h0 + 1
        wqk = np.empty((128, 4, 2, 128), np.float32)
        wv_ = np.empty((128, 4, 2, DH), np.float32)
        for ct in range(4):
            rows = slice(ct * 128, (ct + 1) * 128)
            wqk[:, ct, 0, 0:64] = Wq8[rows, h0 * DH:(h0 + 1) * DH]
            wqk[:, ct, 0, 64:128] = Wk[rows, h1 * DH:(h1 + 1) * DH]
            wqk[:, ct, 1, 0:64] = Wk[rows, h0 * DH:(h0 + 1) * DH]
            wqk[:, ct, 1, 64:128] = Wq8[rows, h1 * DH:(h1 + 1) * DH]
            wv_[:, ct, 0, :] = Wv[rows, h0 * DH:(h0 + 1) * DH]
            wv_[:, ct, 1, :] = Wv[rows, h1 * DH:(h1 + 1) * DH]
        in_maps.append({
            "xT": np.ascontiguousarray(x[b].T).astype(NPBF16),
            "wqk": wqk.astype(NPBF16),
            "wv": wv_.astype(NPBF16),
            "wp": np.ascontiguousarray(
                Wp[h0 * DH:h0 * DH + 2 * DH, :]).astype(NPBF16),
        })

    nc = _get_nc()
    kwargs = {"trace": True} if TRACE else {}
    res = run_bass_kernel_spmd(nc, in_maps, core_ids=list(range(NCORES)), **kwargs)
    global LAST
    LAST = res

    out = np.zeros((2, L, C), np.float32)
    for core in range(NCORES):
        out[core // 4] += res.results[core]["out"]
    out += bv @ Wp + bp
    return out


if __name__ == "__main__":
    d = np.load("inputs.npz")
    expected = np.load("expected.npy")
    got = kernel(**{k: d[k] for k in d.files})
    rel = np.linalg.norm(got - expected) / np.linalg.norm(expected)
    print("Relative error:", rel)


# revision 34
# speedup vs baseline: 1.0830x; 1.0830x over previous
"""Sparse block-attention kernel for Trainium2, 8 NeuronCores.

Problem: y = proj(softmax_masked((x@Wq)(x@Wk)^T/8) @ (x@Wv)) for
B=2, L=3392 (=3*1024 motion + 5*64 text), C=512, H=8 heads, dh=64.

Sharding: 16 (batch, head) pairs over 8 cores -> each core owns one batch
and two adjacent heads (h0=2*(core%4), h1=h0+1). Each core computes its
heads' QKV projections, masked attention, and the partial output
projection y_h0@Wp_h0 + y_h1@Wp_h1 [L, C]; the host sums the 4 partials
per batch.

On-core layout: scores are computed transposed, ST = k q^T [keys, queries],
with both heads packed into the 128 partitions via tensor-engine row
tiling (head0 uses array rows 0-63, head1 rows 64-127). The softmax
denominator comes free from a ones-column appended to V in the AV matmul.
Scores are ~N(0,1) so exp() needs no max subtraction. The block-causal
mask is realized by iterating only allowed key tiles, with trapezoid
query slices and gpsimd affine_select zeroing of diagonal corners
after exp.
"""

import numpy as np
import ml_dtypes

import concourse.bass as bass
import concourse.bacc as bacc
import concourse.mybir as mybir
import concourse.tile as tile
from concourse.bass_utils import run_bass_kernel_spmd
from concourse.masks import make_identity

T = 1024
NTX = 64
L = 3 * T + 5 * NTX  # 3392
C = 512
H = 8
DH = 64
NCORES = 8
N_BLOCKS = 7
BLOCK_Q0 = [0, 512, 1024, 1536, 2048, 2560, 3072]
BLOCK_W = [512, 512, 512, 512, 512, 512, 320]
CHW = [512, 512, 512, 512, 512, 512, 320]  # L chunks (same grid as blocks)
NSUB = [4, 4, 4, 4, 4, 4, 3]  # 128-row subtiles per chunk (last: 128,128,64)
BF16 = mybir.dt.bfloat16
F32 = mybir.dt.float32
NPBF16 = np.dtype(ml_dtypes.bfloat16)


def key_entries(block):
    """Allowed key tiles for a query block: (kstart, klen, qlo, mask).

    The QK/AV matmuls for the tile cover block queries [qlo, W); mask
    in {None,'c','s'} marks a causal/strict triangle on the first 128
    query columns of that slice.
    """
    q0 = BLOCK_Q0[block]
    w = BLOCK_W[block]
    if q0 >= 3 * T:  # text queries attend to all text keys
        return [(3072, 128, 0, None), (3200, 128, 0, None), (3328, 64, 0, None)]
    band = q0 // T
    r0 = q0 % T
    if band == 0:
        specs = [(0, 'c')]
    elif band == 1:
        specs = [(0, 'c'), (T, 's'), (2 * T, 's')]
    else:
        specs = [(0, 'c'), (T, 'c'), (2 * T, 's')]
    out = []
    for kbase, kind in specs:
        for kt in range((r0 + w) // 128):
            local = kt * 128
            if local < r0:
                out.append((kbase + local, 128, 0, None))
            else:
                out.append((kbase + local, 128, local - r0, kind))
    if band == 1:  # text domains 0,2,3,4
        out += [(3072, 64, 0, None), (3200, 128, 0, None), (3328, 64, 0, None)]
    elif band == 2:  # text domains 1,2,3,4
        out += [(3136, 64, 0, None), (3200, 128, 0, None), (3328, 64, 0, None)]
    return out


def _build_nc():
    from contextlib import ExitStack

    nc = bacc.Bacc("TRN2", target_bir_lowering=False, num_devices=NCORES)

    xT_d = nc.dram_tensor("xT", [C, L], BF16, kind="ExternalInput").ap()
    wqk_d = nc.dram_tensor("wqk", [128, 4, 2, 128], BF16, kind="ExternalInput").ap()
    wv_d = nc.dram_tensor("wv", [128, 4, 2, DH], BF16, kind="ExternalInput").ap()
    wp_d = nc.dram_tensor("wp", [128, C], BF16, kind="ExternalInput").ap()
    out_d = nc.dram_tensor("out", [L, C], F32, kind="ExternalOutput").ap()

    with tile.TileContext(nc) as tc:
        with ExitStack() as ctx:
            singles = ctx.enter_context(tc.tile_pool(name="singles", bufs=1))
            psum_st = ctx.enter_context(
                tc.tile_pool(name="psum_st", bufs=2, space="PSUM"))
            psum_yt = ctx.enter_context(
                tc.tile_pool(name="psum_yt", bufs=2, space="PSUM"))
            psum_mm = ctx.enter_context(
                tc.tile_pool(name="psum_mm", bufs=2, space="PSUM"))
            ssb_pool = ctx.enter_context(tc.tile_pool(name="ssb", bufs=8))
            norm_pool = ctx.enter_context(tc.tile_pool(name="norm", bufs=3))
            out_pool = ctx.enter_context(tc.tile_pool(name="outs", bufs=3))

            # ---- weights ----
            wqk = singles.tile([128, 4, 2, 128], BF16, tag="wqk")
            wv = singles.tile([128, 4, 2, DH], BF16, tag="wv")
            wp = singles.tile([128, C], BF16, tag="wp")
            nc.sync.dma_start(out=wqk, in_=wqk_d)

            # ---- constants: 0/1 corner masks (causal, strict) + ones ----
            mcaus = singles.tile([128, 2, 128], BF16, tag="mcaus")
            mstrict = singles.tile([128, 2, 128], BF16, tag="mstrict")
            ones64 = singles.tile([1, 64], F32, tag="ones64")
            nc.gpsimd.memset(ones64, 1.0)
            ident = singles.tile([128, 128], BF16, tag="ident")
            make_identity(nc, ident)
            for mt, base in ((mcaus, 0), (mstrict, -1)):
                nc.gpsimd.memset(mt, 1.0)
                nc.gpsimd.affine_select(
                    out=mt, in_=mt, compare_op=mybir.AluOpType.is_ge,
                    fill=0.0, base=base, pattern=[[0, 2], [1, 128]],
                    channel_multiplier=-1)

            # ---- per-chunk persistent tensors ----
            xt = []   # x^T chunks [128, 4, w]
            sbA = []  # [qT_h0 (parts 0-63); kT_h1 (parts 64-127)] per chunk
            sbB = []  # [kT_h0; qT_h1]
            vaug = []  # [128, g, 130]: cols 0-63 v_h0, 64 ones, 65-128 v_h1, 129 ones
            for c in range(N_BLOCKS):
                w = CHW[c]
                xt.append(singles.tile([128, 4, w], BF16, tag=f"xt{c}", name=f"xt{c}"))
                sbA.append(singles.tile([128, w], BF16, tag=f"sbA{c}", name=f"sbA{c}"))
                sbB.append(singles.tile([128, w], BF16, tag=f"sbB{c}", name=f"sbB{c}"))
                vaug.append(singles.tile([128, NSUB[c], 130], BF16, tag=f"vaug{c}", name=f"vaug{c}"))

            xT_r = xT_d.rearrange("(t p) l -> p t l", p=128)
            for ct in range(4):
                nc.sync.dma_start(out=xt[0][:, ct, :], in_=xT_r[:, ct, 0:512])
            nc.sync.dma_start(out=wv, in_=wv_d)
            nc.sync.dma_start(out=wp, in_=wp_d)
            for c in range(1, N_BLOCKS):
                q0, w = BLOCK_Q0[c], CHW[c]
                nc.sync.dma_start(out=xt[c], in_=xT_r[:, :, q0:q0 + w])

            # ---- QK projections: psum[0:64]=qT/kT h-even, [64:128]=kT/qT h-odd
            def emit_projqk(c):
                w = CHW[c]
                for j, dst in ((0, sbA[c]), (1, sbB[c])):
                    ps = psum_mm.tile([128, 512], F32, tag="mm", name="ps")
                    for ct in range(4):
                        nc.tensor.matmul(
                            ps[:, 0:w], lhsT=wqk[:, ct, j, :], rhs=xt[c][:, ct, :],
                            start=(ct == 0), stop=(ct == 3))
                    nc.vector.tensor_copy(dst[:, :], ps[:, 0:w])

            # ---- V projection: weight-stationary vT matmuls, then PE
            # transposes into the [keys, dh] layout the AV matmul needs ----
            def emit_projv(c):
                w = CHW[c]
                # ones columns 64 and 129 for the softmax-denominator rows
                nc.gpsimd.memset(vaug[c][:, :, 64::65], 1.0)
                vps = psum_mm.tile([128, 512], F32, tag="mm", name="vps")
                for ct in range(4):
                    nc.tensor.matmul(
                        vps[:, 0:w], lhsT=wv[:, ct, :, :], rhs=xt[c][:, ct, :],
                        start=(ct == 0), stop=(ct == 3))
                vts = out_pool.tile([128, 512], BF16, tag="vts", name="vts")
                nc.vector.tensor_copy(vts[:, 0:w], vps[:, 0:w])
                for gi in range(NSUB[c]):
                    rows = min(128, w - gi * 128)
                    tp = psum_mm.tile([128, 128], BF16, tag="mm", name="tp")
                    nc.tensor.transpose(
                        tp[0:rows, :], vts[:, gi * 128:gi * 128 + rows], ident)
                    # scatter [v_h0 | v_h1] -> cols {0:64, 65:129}
                    dst = bass.AP(
                        tensor=vaug[c].tensor,
                        offset=vaug[c].offset + gi * 130,
                        ap=[[vaug[c].ap[0][0], rows], [65, 2], [1, 64]])
                    src = bass.AP(
                        tensor=tp.tensor, offset=tp.offset,
                        ap=[[tp.ap[0][0], rows], [64, 2], [1, 64]])
                    nc.vector.tensor_copy(dst, src)

            # ---- attention + normalize + output projection per query block.
            # Emitted in resumable slices (Tile dependencies follow trace
            # order, so a block's tiles must be emitted after the
            # projection chunks they read).
            astate = {}

            def attn_start(blk):
                astate[blk] = {
                    "yt0": psum_yt.tile([65, 512], F32, tag="yt", name="yt0"),
                    "yt1": psum_yt.tile([65, 512], F32, tag="yt", name="yt1"),
                    "entries": key_entries(blk),
                    "i": 0,
                    "pending": None,
                }

            def _emit_av(blk, item):
                # AV accumulate (row 64 of yt = softmax denominator)
                s = astate[blk]
                w = BLOCK_W[blk]
                nlast = len(s["entries"]) - 1
                i, ssb, pb, klen, qlo, cv, gi = item
                nc.tensor.matmul(
                    s["yt0"][:, qlo:w],
                    lhsT=vaug[cv][pb:pb + klen, gi, 0:65],
                    rhs=ssb[pb:pb + klen, 0, qlo:w],
                    start=(i == 0), stop=(i == nlast), tile_position=(pb, 0))
                nc.tensor.matmul(
                    s["yt1"][:, qlo:w],
                    lhsT=vaug[cv][pb:pb + klen, gi, 65:130],
                    rhs=ssb[pb:pb + klen, 1, qlo:w],
                    start=(i == 0), stop=(i == nlast), tile_position=(pb, 0))

            def attn_tiles(blk, n):
                # emit the next n key tiles (software pipeline: AV trails
                # QK/exp by one tile)
                s = astate[blk]
                w = BLOCK_W[blk]
                for (kstart, klen, qlo, mask) in s["entries"][s["i"]:s["i"] + n]:
                    i = s["i"]
                    ck, ko = kstart // 512, kstart % 512
                    g = kstart // 128
                    cv, gi = g // 4, g % 4
                    pb = kstart % 128  # 0 or 64
                    st = psum_st.tile([128, 2, 512], F32, tag="st", name="st")
                    ssb = ssb_pool.tile([128, 2, 512], BF16, tag="ssb",
                                        name="ssb")
                    # QK^T (scores transposed: [keys, queries]), heads packed
                    nc.tensor.matmul(
                        st[pb:pb + klen, 0, qlo:w],
                        lhsT=sbB[ck][0:64, ko:ko + klen],
                        rhs=sbA[blk][0:64, qlo:w],
                        start=True, stop=True, tile_position=(0, pb))
                    nc.tensor.matmul(
                        st[pb:pb + klen, 1, qlo:w],
                        lhsT=sbA[ck][64:128, ko:ko + klen],
                        rhs=sbB[blk][64:128, qlo:w],
                        start=True, stop=True, tile_position=(64, pb))
                    nc.scalar.activation(
                        ssb[pb:pb + klen, :, qlo:w], st[pb:pb + klen, :, qlo:w],
                        mybir.ActivationFunctionType.Exp)
                    if mask is not None:
                        # zero the blocked triangle of the diagonal corner
                        mt = mcaus if mask == 'c' else mstrict
                        nc.vector.tensor_mul(
                            ssb[pb:pb + klen, :, qlo:qlo + 128],
                            ssb[pb:pb + klen, :, qlo:qlo + 128],
                            mt[pb:pb + klen, :, :])
                    if s["pending"] is not None:
                        _emit_av(blk, s["pending"])
                    s["pending"] = (i, ssb, pb, klen, qlo, cv, gi)
                    s["i"] += 1

            def attn_finish(blk):
                s = astate[blk]
                w = BLOCK_W[blk]
                attn_tiles(blk, len(s["entries"]) - s["i"])
                _emit_av(blk, s["pending"])
                yt0, yt1 = s["yt0"], s["yt1"]

                # normalize: y / l, heads stacked into [128, w] bf16
                ytsb = norm_pool.tile([128, 512], BF16, tag="ytsb")
                h1t = norm_pool.tile([64, 512], BF16, tag="h1t")
                for j, (yt, mdst) in enumerate(((yt0, ytsb[0:64, 0:w]),
                                                (yt1, h1t[:, 0:w]))):
                    rc = norm_pool.tile([1, 512], F32, tag=f"rc{j}", name=f"rc{j}")
                    lsb = norm_pool.tile([1, 512], F32, tag=f"lsb{j}", name=f"lsb{j}")
                    rb = norm_pool.tile([64, 512], F32, tag=f"rb{j}", name=f"rb{j}")
                    nc.vector.tensor_copy(lsb[:, 0:w], yt[64:65, 0:w])
                    nc.vector.reciprocal_approx_fast(rc[:, 0:w], lsb[:, 0:w])
                    nc.gpsimd.partition_broadcast(rb[:, 0:w], rc[:, 0:w])
                    nc.vector.tensor_mul(mdst, yt[0:64, 0:w], rb[:, 0:w])
                # head1 rows into partitions 64-127 (partition shift via DMA)
                nc.sync.dma_start(out=ytsb[64:128, 0:w], in_=h1t[:, 0:w])

                # output projection: one K=128 matmul per 128 queries
                for sub in range(NSUB[blk]):
                    rows = min(128, w - sub * 128)
                    po = psum_mm.tile([128, 512], F32, tag="mm", name="po")
                    nc.tensor.matmul(
                        po[0:rows, :],
                        lhsT=ytsb[:, sub * 128:sub * 128 + rows],
                        rhs=wp[:, :], start=True, stop=True)
                    ost = out_pool.tile([128, 512], F32, tag="ost")
                    nc.vector.tensor_copy(ost[0:rows, :], po[0:rows, :])
                    r0 = BLOCK_Q0[blk] + sub * 128
                    nc.sync.dma_start(out=out_d[r0:r0 + rows, :],
                                      in_=ost[0:rows, :])

            def emit_attn(blk):
                attn_start(blk)
                attn_finish(blk)

            # Interleave projections with attention so the scalar engine
            # (exp, the bottleneck) starts as early as dependencies allow.
            # Big block 3 leads: its band-0 tiles need only chunks 0/1, its
            # band-1 tiles chunks 2/3, so its exp work covers the remaining
            # projections.
            emit_projqk(0)
            emit_projv(0)
            attn_start(0)
            attn_tiles(0, 4)
            emit_projqk(1)
            emit_projv(1)
            attn_finish(0)
            attn_start(1)
            attn_tiles(1, 8)
            emit_projqk(3)
            emit_projv(3)
            attn_finish(1)
            attn_start(3)
            attn_tiles(3, 8)   # band0 key tiles (keys in chunks 0-1)
            emit_projqk(2)
            emit_projv(2)
            attn_tiles(3, 8)   # band1 key tiles (keys in chunks 2-3)
            for c in (6, 4, 5):
                emit_projqk(c)
                emit_projv(c)
            attn_finish(3)
            for blk in (5, 2, 4, 6):
                emit_attn(blk)
    nc.finalize()
    return nc


_NC = None
TRACE = False
LAST = None


def _get_nc():
    global _NC
    if _NC is None:
        _NC = _build_nc()
    return _NC


def kernel(x, Wq, bq, Wk, bk, Wv, bv, Wp, bp, T_motion, N, **_unused):
    x = np.asarray(x, np.float32)
    Wq = np.asarray(Wq, np.float32)
    Wk = np.asarray(Wk, np.float32)
    Wv = np.asarray(Wv, np.float32)
    Wp = np.asarray(Wp, np.float32)
    bq = np.asarray(bq, np.float32)
    bk = np.asarray(bk, np.float32)  # provably softmax-invariant, ignored
    bv = np.asarray(bv, np.float32)
    bp = np.asarray(bp, np.float32)
    assert int(T_motion) == T and int(N) == NTX, "kernel compiled for T=1024,N=64"
    assert x.shape == (2, L, C)
    assert np.allclose(bq, 0.0), "nonzero bq not supported"

    Wq8 = Wq / 8.0
    in_maps = []
    for core in range(NCORES):
        b = core // 4
        h0 = 2 * (core % 4)
        h1 = h0 + 1
        wqk = np.empty((128, 4, 2, 128), np.float32)
        wv_ = np.empty((128, 4, 2, DH), np.float32)
        for ct in range(4):
            rows = slice(ct * 128, (ct + 1) * 128)
            wqk[:, ct, 0, 0:64] = Wq8[rows, h0 * DH:(h0 + 1) * DH]
            wqk[:, ct, 0, 64:128] = Wk[rows, h1 * DH:(h1 + 1) * DH]
            wqk[:, ct, 1, 0:64] = Wk[rows, h0 * DH:(h0 + 1) * DH]
            wqk[:, ct, 1, 64:128] = Wq8[rows, h1 * DH:(h1 + 1) * DH]
            wv_[:, ct, 0, :] = Wv[rows, h0 * DH:(h0 + 1) * DH]
            wv_[:, ct, 1, :] = Wv[rows, h1 * DH:(h1 + 1) * DH]
        in_maps.append({
            "xT": np.ascontiguousarray(x[b].T).astype(NPBF16),
            "wqk": wqk.astype(NPBF16),
            "wv": wv_.astype(NPBF16),
            "wp": np.ascontiguousarray(
                Wp[h0 * DH:h0 * DH + 2 * DH, :]).astype(NPBF16),
        })

    nc = _get_nc()
    kwargs = {"trace": True} if TRACE else {}
    res = run_bass_kernel_spmd(nc, in_maps, core_ids=list(range(NCORES)), **kwargs)
    global LAST
    LAST = res

    out = np.zeros((2, L, C), np.float32)
    for core in range(NCORES):
        out[core // 4] += res.results[core]["out"]
    out += bv @ Wp + bp
    return out


if __name__ == "__main__":
    d = np.load("inputs.npz")
    expected = np.load("expected.npy")
    got = kernel(**{k: d[k] for k in d.files})
    rel = np.linalg.norm(got - expected) / np.linalg.norm(expected)
    print("Relative error:", rel)
